# revision 18
# baseline (speedup 1.0000x reference)
"""Trainium2 Bass kernel for per-position head-mixing attention.

Math (per position p): Qh,Kh,Vh = reshape(q/k/v[p], [16, 64]);
L = Qh @ Kh.T / 8; W = softmax(L, axis=-1); out[p] = W @ Vh.

Strategy:
  * Pure data parallel over 8 cores (4096 positions each).
  * The kernel is HBM-read-bandwidth bound (~215 GB/s per-core sustained
    share).  Everything is organized to keep the read stream saturated
    from t=0 and to drain quickly at the end:
      - 16 chunks of 256 positions, tile-major contiguous DRAM slabs
        (8 KiB per-partition DMA packets, the empirically fastest size).
      - flat software pipeline over all 64 batches (no per-chunk flush),
        4-deep chunk prefetch, per-chunk output stores spread in time.
      - softmax mask rows are built on-chip by SBUF->SBUF doubling DMAs
        (zero HBM traffic), seeded from a single 4.6 KB constant.
  * Host pre-transposes q,k to [d, ...] layout (exact, host-side) and casts
    to bf16 so every device DMA is large and contiguous.
  * Per group of 8 positions, one 73x128x128 matmul computes all 16x16
    logit blocks; 9 extra contraction rows add -C to off-diagonal
    (cross-position) entries so exp() zeroes them - no masking op needed.
    Mask rows live at partitions 64..72 of persistent buffers, written once.
  * exp on ScalarE (scale=1/8 fused), batched over 8 groups.
  * Second matmul W' @ [V | 1] gives numerator and softmax denominator in
    one pass; VectorE reciprocal+multiply normalizes (fp16 out).
  * DMA routing: loads on gpsimd/SWDGE (16-engine spray), mask build +
    output stores on the sync HWDGE ring so they never block loads.
"""

import sys

if "/opt/trn_rl_repo" not in sys.path:
    sys.path.insert(0, "/opt/trn_rl_repo")

from contextlib import ExitStack

import ml_dtypes
import numpy as np

import concourse.bass as bass
from concourse import bacc, mybir, tile
from concourse.bass_utils import run_bass_kernel_spmd

BF16 = mybir.dt.bfloat16
F16 = mybir.dt.float16
F32 = mybir.dt.float32
NPBF16 = ml_dtypes.bfloat16

N_CORES = 8
S_TOT = 4 * 8192          # flattened (batch, seq) positions
H, D = 16, 64             # heads, key size
N_PC = S_TOT // N_CORES   # positions per core
CH = 256                  # positions per chunk
GC = CH // 8              # 8-position groups per chunk (32)
NT = N_PC // CH           # chunks per core (16)
B = 8                     # groups per psum/exp batch
NBC = GC // B             # batches per chunk (4)
NQK = 7                   # qk chunk buffers
PF = 5                    # chunk prefetch depth
C_MASK = 384.0            # diagonal logit boost (C/2=192 exact in bf16)
SCALE = 0.125             # 1/sqrt(64)

_CACHE = {}


def _build_program(nt: int, n_cores: int):
    nc = bacc.Bacc(
        "TRN2", target_bir_lowering=False, debug=False, num_devices=n_cores
    )
    # chunk-major contiguous slabs: [chunk, {q,k}, d, G, H, 8]
    qk = nc.dram_tensor("qk", [nt, 2, 64, GC, H, 8], BF16, kind="ExternalInput").ap()
    mk = nc.dram_tensor("mk", [3, 2, GC * 128], BF16, kind="ExternalInput").ap()
    vr = nc.dram_tensor("vr", [nt, H, 8, GC, 65], BF16, kind="ExternalInput").ap()
    out = nc.dram_tensor("o", [nt, H, 8, GC, D], F16, kind="ExternalOutput").ap()

    QW = GC * 128              # q (or k) columns per chunk buffer
    NB_TOT = nt * NBC          # total batches (64)

    with tile.TileContext(nc) as tc, ExitStack() as ctx:
        qk_pool = ctx.enter_context(tc.tile_pool(name="qk", bufs=1))
        v_pool = ctx.enter_context(tc.tile_pool(name="v", bufs=NQK))
        o_pool = ctx.enter_context(tc.tile_pool(name="o", bufs=4))
        w_pool = ctx.enter_context(tc.tile_pool(name="w", bufs=4))
        r_pool = ctx.enter_context(tc.tile_pool(name="r", bufs=3))
        p1_pool = ctx.enter_context(tc.tile_pool(name="p1", bufs=2, space="PSUM"))
        p2_pool = ctx.enter_context(tc.tile_pool(name="p2", bufs=2, space="PSUM"))

        qk_bufs = [
            qk_pool.tile([67, 2 * QW], BF16, tag=f"qk{j}", name=f"qkbuf{j}")
            for j in range(NQK)
        ]

        # Mask rows: buffer 0 loads directly from DRAM (4 parallel DMAs of
        # 8 KiB/partition); buffers 1..5 copy from buffer 0 SBUF->SBUF in
        # parallel (a single dependency hop - serial DMA chains cost ~3 us
        # of semaphore/dispatch latency per hop).
        b0 = qk_bufs[0]
        mkw = mk.rearrange("p a b -> p (a b)")  # [9, 2*QW]
        for c4 in range(4):
            nc.sync.dma_start(
                b0[64:67, c4 * 2048 : (c4 + 1) * 2048],
                mkw[:, c4 * 2048 : (c4 + 1) * 2048],
            )
        def copy_mask(j):
            buf = qk_bufs[j]
            nc.sync.dma_start(buf[64:67, 0:QW], b0[64:67, 0:QW])
            nc.sync.dma_start(buf[64:67, QW : 2 * QW], b0[64:67, QW : 2 * QW])

        def load_qk(c, split=1):
            buf = qk_bufs[c % NQK]
            src_c = qk[c].rearrange("a p b c d -> p a (b c d)")
            eng = nc.gpsimd if c % 2 == 0 else nc.sync
            for s in range(split):
                lo, hi = s * (QW // split), (s + 1) * (QW // split)
                for side in range(2):
                    eng.dma_start(
                        buf[0:64, side * QW + lo : side * QW + hi],
                        src_c[:, side, lo:hi],
                    )

        def load_v(c, split=1):
            v_t = v_pool.tile([128, GC * 65], BF16)
            src_v = vr[c].rearrange("k p g e -> (k p) (g e)")
            vw = GC * 65
            for s in range(split):
                lo, hi = s * (vw // split), (s + 1) * (vw // split)
                nc.gpsimd.dma_start(v_t[:, lo:hi], src_v[:, lo:hi])
            return v_t

        v_tiles = {}
        for c in range(min(PF, nt)):
            if 1 <= c < NQK:
                copy_mask(c)
            load_qk(c, split=4 if c == 0 else 1)
            v_tiles[c] = load_v(c)

        o_tiles = {}
        pending = []
        for bb in range(NB_TOT + 2):
            if bb < NB_TOT:
                c, bl = bb // NBC, bb % NBC
                if bl == 0:
                    if c + PF < nt:
                        if PF <= c + PF < NQK:
                            copy_mask(c + PF)
                        tail = c + PF >= nt - 2
                        load_qk(c + PF, split=2 if tail else 1)
                        v_tiles[c + PF] = load_v(c + PF, split=2 if tail else 1)
                    o_tiles[c] = o_pool.tile([128, GC * 64], F16, name="o_t")
                qk_t = qk_bufs[c % NQK]
                p1 = p1_pool.tile([128, B * 128], F32)
                for j in range(B):
                    g = bl * B + j
                    nc.tensor.matmul(
                        p1[:, j * 128 : (j + 1) * 128],
                        lhsT=qk_t[:, QW + g * 128 : QW + (g + 1) * 128],
                        rhs=qk_t[:, g * 128 : (g + 1) * 128],
                        start=True,
                        stop=True,
                    )
                w = w_pool.tile([128, B * 128], BF16)
                nc.scalar.activation(
                    w[:], p1[:], mybir.ActivationFunctionType.Exp, scale=SCALE
                )
                pending.append((w, bb))
            if bb >= 2:
                wp, bp = pending.pop(0)
                cp, blp = bp // NBC, bp % NBC
                v_t = v_tiles[cp]
                o_t = o_tiles[cp]
                p2 = p2_pool.tile([128, B * 128], F32)
                for j in range(B):
                    g = blp * B + j
                    nc.tensor.matmul(
                        p2[:, j * 128 : j * 128 + 65],
                        lhsT=wp[:, j * 128 : (j + 1) * 128],
                        rhs=v_t[:, g * 65 : (g + 1) * 65],
                        start=True,
                        stop=True,
                    )
                r = r_pool.tile([128, B], F32)
                p2v = p2[:].rearrange("p (g c) -> p g c", c=128)
                nc.vector.reciprocal(r[:], p2v[:, :, 64])
                rb = r[:].unsqueeze(2).broadcast_to([128, B, 64])
                ov = o_t[:, blp * B * 64 : (blp + 1) * B * 64].rearrange(
                    "p (g c) -> p g c", c=64
                )
                nc.vector.tensor_tensor(
                    ov, p2v[:, :, 0:64], rb, op=mybir.AluOpType.mult
                )
                if blp % 2 == 1:
                    # store the finished half-chunk to spread write traffic
                    oflat = out[cp].rearrange("k p g e -> (k p) (g e)")
                    hw_ = GC * 64 // 2
                    hsel = slice(0, hw_) if blp == 1 else slice(hw_, 2 * hw_)
                    nc.sync.dma_start(oflat[:, hsel], o_t[:, hsel])
                    if blp == NBC - 1:
                        del v_tiles[cp], o_tiles[cp]

    nc.compile()
    return nc


def _prep_qk(qslab: np.ndarray, kslab: np.ndarray, nt: int) -> np.ndarray:
    """Two [nt*CH, 1024] fp32 slabs -> [nt, 2, 64, GC, H, 8] bf16 (chunk-major)."""
    full = np.empty((nt, 2, 64, GC, H, 8), dtype=NPBF16)
    for s, slab in enumerate((qslab, kslab)):
        a = slab.reshape(nt, GC, 8, H, D)           # [c, g, p, h, d]
        full[:, s] = a.transpose(0, 4, 1, 3, 2).astype(NPBF16)
    return full


def _mask_const() -> np.ndarray:
    """[3, 2, GC*128] bf16 mask rows: rank-3 +-1 sign codes.

    Row j holds s_j(p) = +-1 (bit j of position p) on the q side and
    (C/2)*s_j(p) on the k side.  The contraction adds (C/2)*sum_j s_j(p)
    *s_j(p') = (C/2)*(3-2*hamming(p,p')) to each logit: +3C/2 on the
    position diagonal, <= +C/2 off it, so off-diagonal weights are
    suppressed by >= e^{-C/8} after the 1/8-scaled exp (uniform offsets
    cancel in softmax)."""
    m = np.zeros((3, 2, 128), dtype=NPBF16)
    for j in range(3):
        for h in range(H):
            for p in range(8):
                s = 1.0 if (p >> j) & 1 == 0 else -1.0
                m[j, 0, h * 8 + p] = NPBF16(s)
                m[j, 1, h * 8 + p] = NPBF16(C_MASK / 2 * s)
    return np.tile(m, (1, 1, GC))


def _prep_v(slab: np.ndarray, nt: int) -> np.ndarray:
    """[nt*CH, 1024] fp32 -> [nt, H, 8, GC, 65] bf16 with ones column."""
    a = slab.reshape(nt, GC, 8, H, D)
    full = np.empty((nt, H, 8, GC, 65), dtype=NPBF16)
    full[..., :64] = a.transpose(0, 3, 2, 1, 4).astype(NPBF16)
    full[..., 64] = NPBF16(1.0)
    return full


def kernel(q: np.ndarray, k: np.ndarray, v: np.ndarray) -> np.ndarray:
    bshape = q.shape
    qf = np.ascontiguousarray(np.asarray(q, dtype=np.float32)).reshape(S_TOT, H * D)
    kf = np.ascontiguousarray(np.asarray(k, dtype=np.float32)).reshape(S_TOT, H * D)
    vf = np.ascontiguousarray(np.asarray(v, dtype=np.float32)).reshape(S_TOT, H * D)

    key = (NT, N_CORES)
    if key not in _CACHE:
        _CACHE[key] = _build_program(*key)
    nc = _CACHE[key]

    mk = _mask_const()
    in_maps = []
    for c in range(N_CORES):
        s0, s1 = c * N_PC, (c + 1) * N_PC
        in_maps.append(
            {
                "qk": _prep_qk(qf[s0:s1], kf[s0:s1], NT),
                "mk": mk,
                "vr": _prep_v(vf[s0:s1], NT),
            }
        )

    res = run_bass_kernel_spmd(nc, in_maps, core_ids=list(range(N_CORES)))

    out = np.empty((S_TOT, H * D), dtype=np.float32)
    for c in range(N_CORES):
        o = res.results[c]["o"]  # [NT, H, 8, GC, D] fp16
        out[c * N_PC : (c + 1) * N_PC] = (
            o.transpose(0, 3, 2, 1, 4).reshape(N_PC, H * D).astype(np.float32)
        )
    return out.reshape(bshape)


# revision 20
# speedup vs baseline: 1.0767x; 1.0767x over previous
"""Trainium2 Bass kernel for per-position head-mixing attention.

Math (per position p): Qh,Kh,Vh = reshape(q/k/v[p], [16, 64]);
L = Qh @ Kh.T / 8; W = softmax(L, axis=-1); out[p] = W @ Vh.

Strategy:
  * Pure data parallel over 8 cores (4096 positions each).
  * The kernel is HBM-read-bandwidth bound (~215 GB/s per-core sustained
    share).  Everything is organized to keep the read stream saturated
    from t=0 and to drain quickly at the end:
      - 16 chunks of 256 positions, tile-major contiguous DRAM slabs
        (8 KiB per-partition DMA packets, the empirically fastest size).
      - flat software pipeline over all 64 batches (no per-chunk flush),
        4-deep chunk prefetch, per-chunk output stores spread in time.
      - softmax mask rows are built on-chip by SBUF->SBUF doubling DMAs
        (zero HBM traffic), seeded from a single 4.6 KB constant.
  * Host pre-transposes q,k to [d, ...] layout (exact, host-side) and casts
    to bf16 so every device DMA is large and contiguous.
  * Per group of 8 positions, one 73x128x128 matmul computes all 16x16
    logit blocks; 9 extra contraction rows add -C to off-diagonal
    (cross-position) entries so exp() zeroes them - no masking op needed.
    Mask rows live at partitions 64..72 of persistent buffers, written once.
  * exp on ScalarE (scale=1/8 fused), batched over 8 groups.
  * Second matmul W' @ [V | 1] gives numerator and softmax denominator in
    one pass; VectorE reciprocal+multiply normalizes (fp16 out).
  * DMA routing: loads on gpsimd/SWDGE (16-engine spray), mask build +
    output stores on the sync HWDGE ring so they never block loads.
"""

import sys

if "/opt/trn_rl_repo" not in sys.path:
    sys.path.insert(0, "/opt/trn_rl_repo")

from contextlib import ExitStack

import ml_dtypes
import numpy as np

import concourse.bass as bass
from concourse import bacc, mybir, tile
from concourse.bass_utils import run_bass_kernel_spmd

BF16 = mybir.dt.bfloat16
F16 = mybir.dt.float16
F32 = mybir.dt.float32
NPBF16 = ml_dtypes.bfloat16

N_CORES = 8
S_TOT = 4 * 8192          # flattened (batch, seq) positions
H, D = 16, 64             # heads, key size
N_PC = S_TOT // N_CORES   # positions per core
CH = 256                  # positions per chunk
GC = CH // 8              # 8-position groups per chunk (32)
NT = N_PC // CH           # chunks per core (16)
B = 8                     # groups per psum/exp batch
NBC = GC // B             # batches per chunk (4)
NQK = 7                   # qk chunk buffers
PF = 5                    # chunk prefetch depth
C_MASK = 384.0            # diagonal logit boost (C/2=192 exact in bf16)
SCALE = 0.125             # 1/sqrt(64)

_CACHE = {}


def _build_program(nt: int, n_cores: int):
    nc = bacc.Bacc(
        "TRN2", target_bir_lowering=False, debug=False, num_devices=n_cores
    )
    # chunk-major contiguous slabs: [chunk, {q,k}, d, G, H, 8]
    qk = nc.dram_tensor("qk", [nt, 2, 64, GC, H, 8], BF16, kind="ExternalInput").ap()
    mk = nc.dram_tensor("mk", [3, 2, GC * 128], BF16, kind="ExternalInput").ap()
    vr = nc.dram_tensor("vr", [nt, H, 8, GC, 65], BF16, kind="ExternalInput").ap()
    out = nc.dram_tensor("o", [nt, H, 8, GC, D], F16, kind="ExternalOutput").ap()

    QW = GC * 128              # q (or k) columns per chunk buffer
    NB_TOT = nt * NBC          # total batches (64)

    with tile.TileContext(nc) as tc, ExitStack() as ctx:
        qk_pool = ctx.enter_context(tc.tile_pool(name="qk", bufs=1))
        v_pool = ctx.enter_context(tc.tile_pool(name="v", bufs=NQK))
        o_pool = ctx.enter_context(tc.tile_pool(name="o", bufs=4))
        w_pool = ctx.enter_context(tc.tile_pool(name="w", bufs=4))
        r_pool = ctx.enter_context(tc.tile_pool(name="r", bufs=3))
        p1_pool = ctx.enter_context(tc.tile_pool(name="p1", bufs=2, space="PSUM"))
        p2_pool = ctx.enter_context(tc.tile_pool(name="p2", bufs=2, space="PSUM"))

        qk_bufs = [
            qk_pool.tile([67, 2 * QW], BF16, tag=f"qk{j}", name=f"qkbuf{j}")
            for j in range(NQK)
        ]

        # Mask rows: buffer 0 loads directly from DRAM (4 parallel DMAs of
        # 8 KiB/partition); buffers 1..5 copy from buffer 0 SBUF->SBUF in
        # parallel (a single dependency hop - serial DMA chains cost ~3 us
        # of semaphore/dispatch latency per hop).
        b0 = qk_bufs[0]
        mkw = mk.rearrange("p a b -> p (a b)")  # [9, 2*QW]
        for c4 in range(4):
            nc.sync.dma_start(
                b0[64:67, c4 * 2048 : (c4 + 1) * 2048],
                mkw[:, c4 * 2048 : (c4 + 1) * 2048],
            )
        def copy_mask(j):
            buf = qk_bufs[j]
            nc.sync.dma_start(buf[64:67, 0:QW], b0[64:67, 0:QW])
            nc.sync.dma_start(buf[64:67, QW : 2 * QW], b0[64:67, QW : 2 * QW])

        def load_qk(c, split=1):
            buf = qk_bufs[c % NQK]
            src_c = qk[c].rearrange("a p b c d -> p a (b c d)")
            for s in range(split):
                lo, hi = s * (QW // split), (s + 1) * (QW // split)
                for side in range(2):
                    nc.gpsimd.dma_start(
                        buf[0:64, side * QW + lo : side * QW + hi],
                        src_c[:, side, lo:hi],
                    )

        def load_v(c, split=1):
            v_t = v_pool.tile([128, GC * 65], BF16)
            src_v = vr[c].rearrange("k p g e -> (k p) (g e)")
            vw = GC * 65
            for s in range(split):
                lo, hi = s * (vw // split), (s + 1) * (vw // split)
                nc.gpsimd.dma_start(v_t[:, lo:hi], src_v[:, lo:hi])
            return v_t

        v_tiles = {}
        for c in range(min(PF, nt)):
            if c < NQK:
                copy_mask(c)
            load_qk(c, split=4 if c == 0 else 1)
            v_tiles[c] = load_v(c)

        o_tiles = {}
        pending = []
        for bb in range(NB_TOT + 2):
            if bb < NB_TOT:
                c, bl = bb // NBC, bb % NBC
                if bl == 0:
                    if c + PF < nt:
                        if PF <= c + PF < NQK:
                            copy_mask(c + PF)
                        tail = c + PF >= nt - 2
                        load_qk(c + PF, split=2 if tail else 1)
                        v_tiles[c + PF] = load_v(c + PF, split=2 if tail else 1)
                    o_tiles[c] = o_pool.tile([128, GC * 64], F16, name="o_t")
                qk_t = qk_bufs[c % NQK]
                p1 = p1_pool.tile([128, B * 128], F32)
                for j in range(B):
                    g = bl * B + j
                    nc.tensor.matmul(
                        p1[:, j * 128 : (j + 1) * 128],
                        lhsT=qk_t[:, QW + g * 128 : QW + (g + 1) * 128],
                        rhs=qk_t[:, g * 128 : (g + 1) * 128],
                        start=True,
                        stop=True,
                    )
                w = w_pool.tile([128, B * 128], BF16)
                nc.scalar.activation(
                    w[:], p1[:], mybir.ActivationFunctionType.Exp, scale=SCALE
                )
                pending.append((w, bb))
            if bb >= 2:
                wp, bp = pending.pop(0)
                cp, blp = bp // NBC, bp % NBC
                v_t = v_tiles[cp]
                o_t = o_tiles[cp]
                p2 = p2_pool.tile([128, B * 128], F32)
                for j in range(B):
                    g = blp * B + j
                    nc.tensor.matmul(
                        p2[:, j * 128 : j * 128 + 65],
                        lhsT=wp[:, j * 128 : (j + 1) * 128],
                        rhs=v_t[:, g * 65 : (g + 1) * 65],
                        start=True,
                        stop=True,
                    )
                r = r_pool.tile([128, B], F32)
                p2v = p2[:].rearrange("p (g c) -> p g c", c=128)
                nc.vector.reciprocal(r[:], p2v[:, :, 64])
                rb = r[:].unsqueeze(2).broadcast_to([128, B, 64])
                ov = o_t[:, blp * B * 64 : (blp + 1) * B * 64].rearrange(
                    "p (g c) -> p g c", c=64
                )
                nc.vector.tensor_tensor(
                    ov, p2v[:, :, 0:64], rb, op=mybir.AluOpType.mult
                )
                if blp % 2 == 1:
                    # store the finished half-chunk to spread write traffic
                    oflat = out[cp].rearrange("k p g e -> (k p) (g e)")
                    hw_ = GC * 64 // 2
                    hsel = slice(0, hw_) if blp == 1 else slice(hw_, 2 * hw_)
                    nc.sync.dma_start(oflat[:, hsel], o_t[:, hsel])
                    if blp == NBC - 1:
                        del v_tiles[cp], o_tiles[cp]

    nc.compile()
    return nc


def _prep_qk(qslab: np.ndarray, kslab: np.ndarray, nt: int) -> np.ndarray:
    """Two [nt*CH, 1024] fp32 slabs -> [nt, 2, 64, GC, H, 8] bf16 (chunk-major)."""
    full = np.empty((nt, 2, 64, GC, H, 8), dtype=NPBF16)
    for s, slab in enumerate((qslab, kslab)):
        a = slab.reshape(nt, GC, 8, H, D)           # [c, g, p, h, d]
        full[:, s] = a.transpose(0, 4, 1, 3, 2).astype(NPBF16)
    return full


def _mask_const() -> np.ndarray:
    """[3, 2, GC*128] bf16 mask rows: rank-3 +-1 sign codes.

    Row j holds s_j(p) = +-1 (bit j of position p) on the q side and
    (C/2)*s_j(p) on the k side.  The contraction adds (C/2)*sum_j s_j(p)
    *s_j(p') = (C/2)*(3-2*hamming(p,p')) to each logit: +3C/2 on the
    position diagonal, <= +C/2 off it, so off-diagonal weights are
    suppressed by >= e^{-C/8} after the 1/8-scaled exp (uniform offsets
    cancel in softmax)."""
    m = np.zeros((3, 2, 128), dtype=NPBF16)
    for j in range(3):
        for h in range(H):
            for p in range(8):
                s = 1.0 if (p >> j) & 1 == 0 else -1.0
                m[j, 0, h * 8 + p] = NPBF16(s)
                m[j, 1, h * 8 + p] = NPBF16(C_MASK / 2 * s)
    return np.tile(m, (1, 1, GC))


def _prep_v(slab: np.ndarray, nt: int) -> np.ndarray:
    """[nt*CH, 1024] fp32 -> [nt, H, 8, GC, 65] bf16 with ones column."""
    a = slab.reshape(nt, GC, 8, H, D)
    full = np.empty((nt, H, 8, GC, 65), dtype=NPBF16)
    full[..., :64] = a.transpose(0, 3, 2, 1, 4).astype(NPBF16)
    full[..., 64] = NPBF16(1.0)
    return full


def kernel(q: np.ndarray, k: np.ndarray, v: np.ndarray) -> np.ndarray:
    bshape = q.shape
    qf = np.ascontiguousarray(np.asarray(q, dtype=np.float32)).reshape(S_TOT, H * D)
    kf = np.ascontiguousarray(np.asarray(k, dtype=np.float32)).reshape(S_TOT, H * D)
    vf = np.ascontiguousarray(np.asarray(v, dtype=np.float32)).reshape(S_TOT, H * D)

    key = (NT, N_CORES)
    if key not in _CACHE:
        _CACHE[key] = _build_program(*key)
    nc = _CACHE[key]

    mk = _mask_const()
    in_maps = []
    for c in range(N_CORES):
        s0, s1 = c * N_PC, (c + 1) * N_PC
        in_maps.append(
            {
                "qk": _prep_qk(qf[s0:s1], kf[s0:s1], NT),
                "mk": mk,
                "vr": _prep_v(vf[s0:s1], NT),
            }
        )

    res = run_bass_kernel_spmd(nc, in_maps, core_ids=list(range(N_CORES)))

    out = np.empty((S_TOT, H * D), dtype=np.float32)
    for c in range(N_CORES):
        o = res.results[c]["o"]  # [NT, H, 8, GC, D] fp16
        out[c * N_PC : (c + 1) * N_PC] = (
            o.transpose(0, 3, 2, 1, 4).reshape(N_PC, H * D).astype(np.float32)
        )
    return out.reshape(bshape)


# revision 21
# speedup vs baseline: 1.1102x; 1.0312x over previous
"""Trainium2 Bass kernel for per-position head-mixing attention.

Math (per position p): Qh,Kh,Vh = reshape(q/k/v[p], [16, 64]);
L = Qh @ Kh.T / 8; W = softmax(L, axis=-1); out[p] = W @ Vh.

Strategy:
  * Pure data parallel over 8 cores (4096 positions each).
  * The kernel is HBM-read-bandwidth bound (~215 GB/s per-core sustained
    share).  Everything is organized to keep the read stream saturated
    from t=0 and to drain quickly at the end:
      - 16 chunks of 256 positions, tile-major contiguous DRAM slabs
        (8 KiB per-partition DMA packets, the empirically fastest size).
      - flat software pipeline over all 64 batches (no per-chunk flush),
        4-deep chunk prefetch, per-chunk output stores spread in time.
      - softmax mask rows are built on-chip by SBUF->SBUF doubling DMAs
        (zero HBM traffic), seeded from a single 4.6 KB constant.
  * Host pre-transposes q,k to [d, ...] layout (exact, host-side) and casts
    to bf16 so every device DMA is large and contiguous.
  * Per group of 8 positions, one 73x128x128 matmul computes all 16x16
    logit blocks; 9 extra contraction rows add -C to off-diagonal
    (cross-position) entries so exp() zeroes them - no masking op needed.
    Mask rows live at partitions 64..72 of persistent buffers, written once.
  * exp on ScalarE (scale=1/8 fused), batched over 8 groups.
  * Second matmul W' @ [V | 1] gives numerator and softmax denominator in
    one pass; VectorE reciprocal+multiply normalizes (fp16 out).
  * DMA routing: loads on gpsimd/SWDGE (16-engine spray), mask build +
    output stores on the sync HWDGE ring so they never block loads.
"""

import sys

if "/opt/trn_rl_repo" not in sys.path:
    sys.path.insert(0, "/opt/trn_rl_repo")

from contextlib import ExitStack

import ml_dtypes
import numpy as np

import concourse.bass as bass
from concourse import bacc, mybir, tile
from concourse.bass_utils import run_bass_kernel_spmd

BF16 = mybir.dt.bfloat16
F16 = mybir.dt.float16
F32 = mybir.dt.float32
NPBF16 = ml_dtypes.bfloat16

N_CORES = 8
S_TOT = 4 * 8192          # flattened (batch, seq) positions
H, D = 16, 64             # heads, key size
N_PC = S_TOT // N_CORES   # positions per core
CH = 256                  # positions per chunk
GC = CH // 8              # 8-position groups per chunk (32)
NT = N_PC // CH           # chunks per core (16)
B = 8                     # groups per psum/exp batch
NBC = GC // B             # batches per chunk (4)
NQK = 7                   # qk chunk buffers
PF = 4                    # chunk prefetch depth
C_MASK = 384.0            # diagonal logit boost (C/2=192 exact in bf16)
SCALE = 0.125             # 1/sqrt(64)

_CACHE = {}


def _build_program(nt: int, n_cores: int):
    nc = bacc.Bacc(
        "TRN2", target_bir_lowering=False, debug=False, num_devices=n_cores
    )
    # chunk-major contiguous slabs: [chunk, {q,k}, d, G, H, 8]
    qk = nc.dram_tensor("qk", [nt, 2, 64, GC, H, 8], BF16, kind="ExternalInput").ap()
    mk = nc.dram_tensor("mk", [3, 2, GC * 128], BF16, kind="ExternalInput").ap()
    vr = nc.dram_tensor("vr", [nt, H, 8, GC, 65], BF16, kind="ExternalInput").ap()
    out = nc.dram_tensor("o", [nt, H, 8, GC, D], F16, kind="ExternalOutput").ap()

    QW = GC * 128              # q (or k) columns per chunk buffer
    NB_TOT = nt * NBC          # total batches (64)

    with tile.TileContext(nc) as tc, ExitStack() as ctx:
        qk_pool = ctx.enter_context(tc.tile_pool(name="qk", bufs=1))
        v_pool = ctx.enter_context(tc.tile_pool(name="v", bufs=NQK))
        o_pool = ctx.enter_context(tc.tile_pool(name="o", bufs=6))
        w_pool = ctx.enter_context(tc.tile_pool(name="w", bufs=4))
        r_pool = ctx.enter_context(tc.tile_pool(name="r", bufs=3))
        p1_pool = ctx.enter_context(tc.tile_pool(name="p1", bufs=2, space="PSUM"))
        p2_pool = ctx.enter_context(tc.tile_pool(name="p2", bufs=2, space="PSUM"))

        qk_bufs = [
            qk_pool.tile([67, 2 * QW], BF16, tag=f"qk{j}", name=f"qkbuf{j}")
            for j in range(NQK)
        ]

        # Mask rows: buffer 0 loads directly from DRAM (4 parallel DMAs of
        # 8 KiB/partition); buffers 1..5 copy from buffer 0 SBUF->SBUF in
        # parallel (a single dependency hop - serial DMA chains cost ~3 us
        # of semaphore/dispatch latency per hop).
        b0 = qk_bufs[0]
        mkw = mk.rearrange("p a b -> p (a b)")  # [9, 2*QW]
        for c4 in range(4):
            nc.sync.dma_start(
                b0[64:67, c4 * 2048 : (c4 + 1) * 2048],
                mkw[:, c4 * 2048 : (c4 + 1) * 2048],
            )
        def copy_mask(j):
            buf = qk_bufs[j]
            nc.sync.dma_start(buf[64:67, 0:QW], b0[64:67, 0:QW])
            nc.sync.dma_start(buf[64:67, QW : 2 * QW], b0[64:67, QW : 2 * QW])

        def load_qk(c, split=1):
            buf = qk_bufs[c % NQK]
            src_c = qk[c].rearrange("a p b c d -> p a (b c d)")
            for s in range(split):
                lo, hi = s * (QW // split), (s + 1) * (QW // split)
                for side in range(2):
                    nc.gpsimd.dma_start(
                        buf[0:64, side * QW + lo : side * QW + hi],
                        src_c[:, side, lo:hi],
                    )

        def load_v(c, split=1):
            v_t = v_pool.tile([128, GC * 65], BF16)
            src_v = vr[c].rearrange("k p g e -> (k p) (g e)")
            vw = GC * 65
            for s in range(split):
                lo, hi = s * (vw // split), (s + 1) * (vw // split)
                nc.gpsimd.dma_start(v_t[:, lo:hi], src_v[:, lo:hi])
            return v_t

        v_tiles = {}
        for c in range(min(PF, nt)):
            if c < NQK:
                copy_mask(c)
            load_qk(c, split=4 if c == 0 else 1)
            v_tiles[c] = load_v(c)

        o_tiles = {}
        pending = []
        for bb in range(NB_TOT + 2):
            if bb < NB_TOT:
                c, bl = bb // NBC, bb % NBC
                if bl == 0:
                    if c + PF < nt:
                        if PF <= c + PF < NQK:
                            copy_mask(c + PF)
                        tail = c + PF >= nt - 2
                        load_qk(c + PF, split=4 if tail else 1)
                        v_tiles[c + PF] = load_v(c + PF, split=4 if tail else 1)
                    o_tiles[c] = o_pool.tile([128, GC * 64], F16, name="o_t")
                qk_t = qk_bufs[c % NQK]
                p1 = p1_pool.tile([128, B * 128], F32)
                for j in range(B):
                    g = bl * B + j
                    nc.tensor.matmul(
                        p1[:, j * 128 : (j + 1) * 128],
                        lhsT=qk_t[:, QW + g * 128 : QW + (g + 1) * 128],
                        rhs=qk_t[:, g * 128 : (g + 1) * 128],
                        start=True,
                        stop=True,
                    )
                w = w_pool.tile([128, B * 128], BF16)
                nc.scalar.activation(
                    w[:], p1[:], mybir.ActivationFunctionType.Exp, scale=SCALE
                )
                pending.append((w, bb))
            if bb >= 2:
                wp, bp = pending.pop(0)
                cp, blp = bp // NBC, bp % NBC
                v_t = v_tiles[cp]
                o_t = o_tiles[cp]
                p2 = p2_pool.tile([128, B * 128], F32)
                for j in range(B):
                    g = blp * B + j
                    nc.tensor.matmul(
                        p2[:, j * 128 : j * 128 + 65],
                        lhsT=wp[:, j * 128 : (j + 1) * 128],
                        rhs=v_t[:, g * 65 : (g + 1) * 65],
                        start=True,
                        stop=True,
                    )
                r = r_pool.tile([128, B], F32)
                p2v = p2[:].rearrange("p (g c) -> p g c", c=128)
                nc.vector.reciprocal(r[:], p2v[:, :, 64])
                rb = r[:].unsqueeze(2).broadcast_to([128, B, 64])
                ov = o_t[:, blp * B * 64 : (blp + 1) * B * 64].rearrange(
                    "p (g c) -> p g c", c=64
                )
                nc.vector.tensor_tensor(
                    ov, p2v[:, :, 0:64], rb, op=mybir.AluOpType.mult
                )
                if blp % 2 == 1:
                    # store the finished half-chunk to spread write traffic
                    oflat = out[cp].rearrange("k p g e -> (k p) (g e)")
                    hw_ = GC * 64 // 2
                    hsel = slice(0, hw_) if blp == 1 else slice(hw_, 2 * hw_)
                    nc.sync.dma_start(oflat[:, hsel], o_t[:, hsel])
                    if blp == NBC - 1:
                        del v_tiles[cp], o_tiles[cp]

    nc.compile()
    return nc


def _prep_qk(qslab: np.ndarray, kslab: np.ndarray, nt: int) -> np.ndarray:
    """Two [nt*CH, 1024] fp32 slabs -> [nt, 2, 64, GC, H, 8] bf16 (chunk-major)."""
    full = np.empty((nt, 2, 64, GC, H, 8), dtype=NPBF16)
    for s, slab in enumerate((qslab, kslab)):
        a = slab.reshape(nt, GC, 8, H, D)           # [c, g, p, h, d]
        full[:, s] = a.transpose(0, 4, 1, 3, 2).astype(NPBF16)
    return full


def _mask_const() -> np.ndarray:
    """[3, 2, GC*128] bf16 mask rows: rank-3 +-1 sign codes.

    Row j holds s_j(p) = +-1 (bit j of position p) on the q side and
    (C/2)*s_j(p) on the k side.  The contraction adds (C/2)*sum_j s_j(p)
    *s_j(p') = (C/2)*(3-2*hamming(p,p')) to each logit: +3C/2 on the
    position diagonal, <= +C/2 off it, so off-diagonal weights are
    suppressed by >= e^{-C/8} after the 1/8-scaled exp (uniform offsets
    cancel in softmax)."""
    m = np.zeros((3, 2, 128), dtype=NPBF16)
    for j in range(3):
        for h in range(H):
            for p in range(8):
                s = 1.0 if (p >> j) & 1 == 0 else -1.0
                m[j, 0, h * 8 + p] = NPBF16(s)
                m[j, 1, h * 8 + p] = NPBF16(C_MASK / 2 * s)
    return np.tile(m, (1, 1, GC))


def _prep_v(slab: np.ndarray, nt: int) -> np.ndarray:
    """[nt*CH, 1024] fp32 -> [nt, H, 8, GC, 65] bf16 with ones column."""
    a = slab.reshape(nt, GC, 8, H, D)
    full = np.empty((nt, H, 8, GC, 65), dtype=NPBF16)
    full[..., :64] = a.transpose(0, 3, 2, 1, 4).astype(NPBF16)
    full[..., 64] = NPBF16(1.0)
    return full


def kernel(q: np.ndarray, k: np.ndarray, v: np.ndarray) -> np.ndarray:
    bshape = q.shape
    qf = np.ascontiguousarray(np.asarray(q, dtype=np.float32)).reshape(S_TOT, H * D)
    kf = np.ascontiguousarray(np.asarray(k, dtype=np.float32)).reshape(S_TOT, H * D)
    vf = np.ascontiguousarray(np.asarray(v, dtype=np.float32)).reshape(S_TOT, H * D)

    key = (NT, N_CORES)
    if key not in _CACHE:
        _CACHE[key] = _build_program(*key)
    nc = _CACHE[key]

    mk = _mask_const()
    in_maps = []
    for c in range(N_CORES):
        s0, s1 = c * N_PC, (c + 1) * N_PC
        in_maps.append(
            {
                "qk": _prep_qk(qf[s0:s1], kf[s0:s1], NT),
                "mk": mk,
                "vr": _prep_v(vf[s0:s1], NT),
            }
        )

    res = run_bass_kernel_spmd(nc, in_maps, core_ids=list(range(N_CORES)))

    out = np.empty((S_TOT, H * D), dtype=np.float32)
    for c in range(N_CORES):
        o = res.results[c]["o"]  # [NT, H, 8, GC, D] fp16
        out[c * N_PC : (c + 1) * N_PC] = (
            o.transpose(0, 3, 2, 1, 4).reshape(N_PC, H * D).astype(np.float32)
        )
    return out.reshape(bshape)
